# revision 9
# baseline (speedup 1.0000x reference)
"""DAEGC (2-layer GAT + clustering head) forward pass on 8 Trainium2 NeuronCores.

Sharding: row-wise over the N x N attention/adjacency matrices — core i owns
rows [i*1024, (i+1)*1024) of adj, M, attention, h', A_pred.  h1/h2/z are
all-gathered ([N, d], small); W/a/cluster params replicated.

Algorithm notes:
 - softmax without max-subtraction: scores are bounded (|s| < ~25), masked
   entries multiply by adj (0/1) after exp, so masked exp = 0 exactly and
   denominator = sum of masked exps.  h' = (expS @ G) / den.
 - attention aggregation computed transposed: h'^T = G^T @ expS^T with expS
   tiles transposed on the PE; contraction (neighbor j) stays on the
   partition axis with a 512-wide moving free dim.
 - matmul operands are float32r (rounded fp32, ~2e-4 rel) for 4x PE rate.
"""
import sys
sys.path.insert(0, '/opt/trn_rl_repo')
import contextlib
import numpy as np

import concourse.bass as bass
import concourse.tile as tile
import concourse.mybir as mybir
from concourse import bacc
from concourse.masks import make_identity
import concourse.bass_utils as bu

f32 = mybir.dt.float32
f32r = mybir.dt.float32r
i32 = mybir.dt.int32
AF = mybir.ActivationFunctionType
OP = mybir.AluOpType

NC = 8            # cores
N = 8192          # nodes
F = 512           # input features
H = 256           # hidden
E = 64            # embed
KC = 16           # clusters
R = N // NC       # rows per core = 1024
RT = R // 128     # row tiles per core = 8
PW = 512          # attention panel width (j)
NPAN = N // PW    # 16 panels
RG = [list(range(NC))]

_cache = {}


def _attention_layer(nc, tc, ctx, consts, out_pool, lay):
    """Emit one GAT attention layer; returns (softmax-normalized + ELU) h'^T
    tile [fp, fcs, 1024] f32 allocated from out_pool."""
    ident, ones_r = consts['ident'], consts['ones_r']
    Fg, tag = lay['Fg'], lay['tag']
    fcs = (Fg + 127) // 128          # feature chunks (2 for L1, 1 for L2)
    fp = min(Fg, 128)                # partitions per chunk
    G_re = lay['G'].ap().rearrange("(b p) f -> p b f", p=128)

    pan_pool = ctx.enter_context(tc.tile_pool(name=f"pan{tag}", bufs=2))
    sc_pool = ctx.enter_context(tc.tile_pool(name=f"sc{tag}", bufs=2))
    nrm_pool = ctx.enter_context(tc.tile_pool(name=f"nrm{tag}", bufs=1))
    et_pool = ctx.enter_context(tc.tile_pool(name=f"et{tag}", bufs=2))
    ps_acc = ctx.enter_context(tc.tile_pool(name=f"psa{tag}", bufs=1, space="PSUM"))
    ps_t = ctx.enter_context(tc.tile_pool(name=f"pst{tag}", bufs=2, space="PSUM"))

    # persistent accumulators: h'^T [fp, 512] per (fc, rh); den [1, 512] per rh
    ph = [[ps_acc.tile([fp, 512], f32, tag=f"ph{fc}{rh}", name=f"ph{tag}{fc}{rh}")
           for rh in range(2)] for fc in range(fcs)]
    pden = [ps_acc.tile([1, 512], f32, tag=f"pden{rh}", name=f"pden{tag}{rh}")
            for rh in range(2)]

    for p in range(NPAN):
        j0 = p * PW
        Mp = pan_pool.tile([128, RT, PW], f32, tag="Mp")
        Ap = pan_pool.tile([128, RT, PW], i32, tag="Ap")
        Gp = pan_pool.tile([128, 4, Fg], f32r, tag="Gp")
        nc.sync.dma_start(out=Mp, in_=lay['m_re'][:, :, j0:j0 + PW])
        nc.sync.dma_start(out=Ap, in_=lay['adj_re'][:, :, j0:j0 + PW])
        nc.sync.dma_start(out=Gp, in_=G_re[:, p * 4:(p + 1) * 4, :])
        ANp = pan_pool.tile([128, PW], f32, tag="ANp")
        nc.sync.dma_start(out=ANp, in_=bass.AP(tensor=lay['an_full'], offset=j0,
                                               ap=[[0, 128], [1, PW]]))

        eT = et_pool.tile([128, 4, 1024], f32r, tag="eT")
        for rt in range(RT):
            t = sc_pool.tile([128, PW], f32, tag="t")
            nc.vector.scalar_tensor_tensor(
                out=t, in0=ANp, scalar=lay['as_cols'][:, rt:rt + 1],
                in1=Mp[:, rt], op0=OP.add, op1=OP.mult)
            v = sc_pool.tile([128, PW], f32, tag="v")
            nc.vector.scalar_tensor_tensor(
                out=v, in0=t, scalar=0.2, in1=t, op0=OP.mult, op1=OP.max)
            e = sc_pool.tile([128, PW], f32, tag="e")
            nc.scalar.activation(e, v, AF.Exp)
            em = sc_pool.tile([128, PW], f32, tag="em")
            nc.vector.scalar_tensor_tensor(
                out=em, in0=e, scalar=1.0, in1=Ap[:, rt],
                op0=OP.mult, op1=OP.mult)
            for jh in range(2):
                pt = ps_t.tile([128, 256], f32, tag="pt")
                nc.tensor.transpose(pt[:, 0:128], em[:, jh * 256:jh * 256 + 128], ident)
                nc.tensor.transpose(pt[:, 128:256], em[:, jh * 256 + 128:jh * 256 + 256], ident)
                nc.scalar.activation(
                    eT[:, 2 * jh:2 * jh + 2, rt * 128:rt * 128 + 128],
                    pt.rearrange("p (b q) -> p b q", b=2), AF.Copy)

        first, last = (p == 0), (p == NPAN - 1)
        for jb in range(4):
            for fc in range(fcs):
                for rh in range(2):
                    nc.tensor.matmul(
                        ph[fc][rh],
                        lhsT=Gp[:, jb, fc * 128:fc * 128 + fp],
                        rhs=eT[:, jb, rh * 512:rh * 512 + 512],
                        start=(first and jb == 0), stop=(last and jb == 3))
            for rh in range(2):
                nc.tensor.matmul(
                    pden[rh], lhsT=ones_r,
                    rhs=eT[:, jb, rh * 512:rh * 512 + 512],
                    start=(first and jb == 0), stop=(last and jb == 3))

    # reciprocal of softmax denominators -> broadcast along feature partitions
    rden_row = nrm_pool.tile([1, 1024], f32, tag="rden_row")
    for rh in range(2):
        nc.vector.reciprocal(rden_row[:, rh * 512:rh * 512 + 512], pden[rh])
    RDEN = nrm_pool.tile([fp, 1024], f32, tag="RDEN")
    nc.gpsimd.partition_broadcast(RDEN, rden_row)

    # normalize (softmax divide) + ELU -> houtT [fp, fcs, 1024] f32
    houtT = out_pool.tile([fp, fcs, 1024], f32, tag=f"houtT{tag}")
    for fc in range(fcs):
        for rh in range(2):
            sl = slice(rh * 512, rh * 512 + 512)
            hn = nrm_pool.tile([fp, 512], f32, tag="hn")
            nc.vector.tensor_tensor(hn, ph[fc][rh], RDEN[:, sl], op=OP.mult)
            mn = nrm_pool.tile([fp, 512], f32, tag="mn")
            nc.vector.tensor_scalar(out=mn, in0=hn, scalar1=0.0, scalar2=None, op0=OP.min)
            ex = nrm_pool.tile([fp, 512], f32, tag="ex")
            nc.scalar.activation(ex, mn, AF.Exp)
            rl = nrm_pool.tile([fp, 512], f32, tag="rl")
            nc.vector.tensor_relu(rl, hn)
            nc.vector.scalar_tensor_tensor(
                out=houtT[:, fc, sl], in0=ex, scalar=-1.0, in1=rl,
                op0=OP.add, op1=OP.add)
    return houtT


def _build():
    nc = bacc.Bacc("TRN2", target_bir_lowering=False, debug=False, num_devices=NC)

    x_d = nc.dram_tensor("x", [R, F], f32, kind="ExternalInput")
    adj_d = nc.dram_tensor("adj", [R, N], i32, kind="ExternalInput")
    m_d = nc.dram_tensor("m", [R, N], f32, kind="ExternalInput")
    w1_d = nc.dram_tensor("w1", [F, H], f32r, kind="ExternalInput")
    a1s_d = nc.dram_tensor("a1s", [1, H], f32r, kind="ExternalInput")
    a1n_d = nc.dram_tensor("a1n", [1, H], f32r, kind="ExternalInput")
    w2_d = nc.dram_tensor("w2", [H, E], f32r, kind="ExternalInput")
    a2s_d = nc.dram_tensor("a2s", [1, E], f32r, kind="ExternalInput")
    a2n_d = nc.dram_tensor("a2n", [1, E], f32r, kind="ExternalInput")
    cc_d = nc.dram_tensor("cc", [KC, E], f32, kind="ExternalInput")

    apred_d = nc.dram_tensor("apred", [R, N], f32, kind="ExternalOutput")
    z_d = nc.dram_tensor("z", [R, E], f32, kind="ExternalOutput")
    q_d = nc.dram_tensor("q", [R, KC], f32, kind="ExternalOutput")

    adj_re = adj_d.ap().rearrange("(a p) j -> p a j", p=128)
    m_re = m_d.ap().rearrange("(a p) j -> p a j", p=128)
    x_re = x_d.ap().rearrange("(a p) f -> p a f", p=128)
    apred_re = apred_d.ap().rearrange("(a p) j -> p a j", p=128)
    z_re = z_d.ap().rearrange("(a p) e -> p a e", p=128)
    q_re = q_d.ap().rearrange("(a p) k -> p a k", p=128)

    with tile.TileContext(nc, num_cores=NC) as tc, contextlib.ExitStack() as top:
        const = top.enter_context(tc.tile_pool(name="const", bufs=1))
        dram = top.enter_context(tc.tile_pool(name="dram", bufs=1, space="DRAM"))
        keep = top.enter_context(tc.tile_pool(name="keep", bufs=1))

        ident = const.tile([128, 128], f32)
        make_identity(nc, ident)
        ones_f = const.tile([128, 1], f32)
        nc.vector.memset(ones_f, 1.0)
        ones_r = const.tile([128, 1], f32r)
        nc.scalar.activation(ones_r, ones_f, AF.Copy)
        consts = dict(ident=ident, ones_r=ones_r)

        # ---- params to SBUF (f32r for matmul operands) ----
        w1_r = const.tile([128, 4, H], f32r)
        nc.sync.dma_start(out=w1_r, in_=w1_d.ap().rearrange("(c p) h -> p c h", p=128))
        w2_r = const.tile([128, 2, E], f32r)
        nc.sync.dma_start(out=w2_r, in_=w2_d.ap().rearrange("(c p) e -> p c e", p=128))

        def vec_cols(d, n):  # DRAM [1, n] -> SBUF column layout [min(n,128), n/p] f32r
            p = min(n, 128)
            c = n // p
            r = const.tile([p, c], f32r, tag=f"vcr{d.name}")
            nc.sync.dma_start(out=r, in_=bass.AP(tensor=d, offset=0, ap=[[1, p], [p, c]]))
            return r

        a1s_cols = vec_cols(a1s_d, H)   # [128, 2]
        a1n_cols = vec_cols(a1n_d, H)   # [128, 2]
        a2s_cols = vec_cols(a2s_d, E)   # [64, 1]
        a2n_cols = vec_cols(a2n_d, E)   # [64, 1]

        cc_f = const.tile([KC, E], f32)
        nc.sync.dma_start(out=cc_f, in_=cc_d.ap())

        # DRAM bounces for collectives
        h1_b = dram.tile([R, H], f32r)
        h1_full = dram.tile([N, H], f32r, addr_space="Shared")
        an1_b = dram.tile([1, R], f32)
        an1_full = dram.tile([NC, R], f32, addr_space="Shared")
        h2_b = dram.tile([R, E], f32r)
        h2_full = dram.tile([N, E], f32r, addr_space="Shared")
        an2_b = dram.tile([1, R], f32)
        an2_full = dram.tile([NC, R], f32, addr_space="Shared")
        z_b = dram.tile([R, E], f32)
        z_full = dram.tile([N, E], f32, addr_space="Shared")

        # ================= stage A: h1 = x @ W1, attention vectors =================
        with contextlib.ExitStack() as sa:
            sA = sa.enter_context(tc.tile_pool(name="sA", bufs=2))
            psA = sa.enter_context(tc.tile_pool(name="psA", bufs=2, space="PSUM"))

            xs = sA.tile([128, RT, F], f32, tag="xs")
            nc.sync.dma_start(out=xs, in_=x_re)
            xT = sA.tile([128, 4, R], f32r, tag="xT")
            for rt in range(RT):
                ptx = psA.tile([128, 512], f32, tag="psAbig")
                for fc in range(4):
                    nc.tensor.transpose(ptx[:, fc * 128:(fc + 1) * 128],
                                        xs[:, rt, fc * 128:(fc + 1) * 128], ident)
                nc.scalar.activation(xT[:, :, rt * 128:rt * 128 + 128],
                                     ptx.rearrange("p (c q) -> p c q", c=4), AF.Copy)

            h1T = sA.tile([128, 2, R], f32r, tag="h1T")
            for hh in range(2):
                for rh in range(2):
                    ph1 = psA.tile([128, 512], f32, tag="psAbig")
                    for fc in range(4):
                        nc.tensor.matmul(ph1, lhsT=w1_r[:, fc, hh * 128:(hh + 1) * 128],
                                         rhs=xT[:, fc, rh * 512:rh * 512 + 512],
                                         start=(fc == 0), stop=(fc == 3))
                    nc.scalar.activation(h1T[:, hh, rh * 512:rh * 512 + 512], ph1, AF.Copy)

            # h1 row-major (for AllGather) — f32r
            h1_sb = sA.tile([128, RT, H], f32r, tag="h1_sb")
            for rt in range(RT):
                pth = psA.tile([128, 256], f32, tag="psAbig")
                for hh in range(2):
                    nc.tensor.transpose(pth[:, hh * 128:(hh + 1) * 128],
                                        h1T[:, hh, rt * 128:rt * 128 + 128].bitcast(f32), ident)
                nc.scalar.activation(h1_sb[:, rt, :],
                                     pth.rearrange("p (c q) -> p c q", c=2), AF.Copy)
            nc.sync.dma_start(out=h1_b.rearrange("(a p) h -> p a h", p=128), in_=h1_sb)

            # an1/as1 rows [1, R] via matmul, as1 -> columns via PE transpose
            an1_row = sA.tile([1, R], f32, tag="an1_row")
            as1_row = sA.tile([1, R], f32, tag="as1_row")
            for (vec, row) in ((a1n_cols, an1_row), (a1s_cols, as1_row)):
                for rh in range(2):
                    pa = psA.tile([1, 512], f32, tag="psAsm")
                    for hh in range(2):
                        nc.tensor.matmul(pa, lhsT=vec[:, hh:hh + 1],
                                         rhs=h1T[:, hh, rh * 512:rh * 512 + 512],
                                         start=(hh == 0), stop=(hh == 1))
                    nc.scalar.activation(row[:, rh * 512:rh * 512 + 512], pa, AF.Copy)
            nc.sync.dma_start(out=an1_b, in_=an1_row)

            as1_cols = keep.tile([128, RT], f32, tag="as1_cols")
            pas = psA.tile([128, RT], f32, tag="psAsm")
            for rt in range(RT):
                nc.tensor.transpose(pas[:, rt:rt + 1],
                                    as1_row[0:1, rt * 128:rt * 128 + 128], ident[0:1, 0:1])
            nc.scalar.activation(as1_cols, pas, AF.Copy)

            nc.gpsimd.collective_compute("AllGather", OP.bypass, replica_groups=RG,
                                         ins=[h1_b.opt()], outs=[h1_full.opt()])
            nc.gpsimd.collective_compute("AllGather", OP.bypass, replica_groups=RG,
                                         ins=[an1_b.opt()], outs=[an1_full.opt()])


        # ================= stage B: layer-1 attention =================
        with contextlib.ExitStack() as sb1:
            sb1_out = sb1.enter_context(tc.tile_pool(name="ob1", bufs=1))
            x2T = _attention_layer(nc, tc, sb1, consts, sb1_out, dict(
                tag="L1", G=h1_full.tensor, an_full=an1_full.tensor, as_cols=as1_cols,
                m_re=m_re, adj_re=adj_re, Fg=H))
            x2T_r = keep.tile([128, 2, R], f32r, tag="x2T_r")
            nc.scalar.activation(x2T_r, x2T, AF.Copy)

        # ================= stage C: h2 = x2 @ W2, attention vectors =================
        with contextlib.ExitStack() as sc:
            sC = sc.enter_context(tc.tile_pool(name="sC", bufs=2))
            psC = sc.enter_context(tc.tile_pool(name="psC", bufs=2, space="PSUM"))

            h2T = sC.tile([E, R], f32r, tag="h2T")
            for rh in range(2):
                ph2 = psC.tile([E, 512], f32, tag="psCbig")
                for fc in range(2):
                    nc.tensor.matmul(ph2, lhsT=w2_r[:, fc, :],
                                     rhs=x2T_r[:, fc, rh * 512:rh * 512 + 512],
                                     start=(fc == 0), stop=(fc == 1))
                nc.scalar.activation(h2T[:, rh * 512:rh * 512 + 512], ph2, AF.Copy)

            h2_sb = sC.tile([128, RT, E], f32r, tag="h2_sb")
            for rt in range(RT):
                pth2 = psC.tile([128, E], f32, tag="psCbig")
                nc.tensor.transpose(pth2, h2T[:, rt * 128:rt * 128 + 128].bitcast(f32), ident[0:E, 0:E])
                nc.scalar.activation(h2_sb[:, rt, :], pth2, AF.Copy)
            nc.sync.dma_start(out=h2_b.rearrange("(a p) e -> p a e", p=128), in_=h2_sb)

            an2_row = sC.tile([1, R], f32, tag="an2_row")
            as2_row = sC.tile([1, R], f32, tag="as2_row")
            for (vec, row) in ((a2n_cols, an2_row), (a2s_cols, as2_row)):
                for rh in range(2):
                    pa2 = psC.tile([1, 512], f32, tag="psCsm")
                    nc.tensor.matmul(pa2, lhsT=vec, rhs=h2T[:, rh * 512:rh * 512 + 512],
                                     start=True, stop=True)
                    nc.scalar.activation(row[:, rh * 512:rh * 512 + 512], pa2, AF.Copy)
            nc.sync.dma_start(out=an2_b, in_=an2_row)

            as2_cols = keep.tile([128, RT], f32, tag="as2_cols")
            pas2 = psC.tile([128, RT], f32, tag="psCsm")
            for rt in range(RT):
                nc.tensor.transpose(pas2[:, rt:rt + 1],
                                    as2_row[0:1, rt * 128:rt * 128 + 128], ident[0:1, 0:1])
            nc.scalar.activation(as2_cols, pas2, AF.Copy)

            nc.gpsimd.collective_compute("AllGather", OP.bypass, replica_groups=RG,
                                         ins=[h2_b.opt()], outs=[h2_full.opt()])
            nc.gpsimd.collective_compute("AllGather", OP.bypass, replica_groups=RG,
                                         ins=[an2_b.opt()], outs=[an2_full.opt()])


        # ================= stage D: layer-2 attention =================
        with contextlib.ExitStack() as sb2:
            g2T = _attention_layer(nc, tc, sb2, consts, keep, dict(
                tag="L2", G=h2_full.tensor, an_full=an2_full.tensor, as_cols=as2_cols,
                m_re=m_re, adj_re=adj_re, Fg=E))
        g2T_f = g2T.rearrange("p one r -> p (one r)")  # [64, 1024]

        # ================= stage E: z, q, A_pred =================
        with contextlib.ExitStack() as se:
            sE = se.enter_context(tc.tile_pool(name="sE", bufs=2))
            sE1 = se.enter_context(tc.tile_pool(name="sE1", bufs=1))
            psE = se.enter_context(tc.tile_pool(name="psE", bufs=2, space="PSUM"))

            # row L2 norms: sum over e (partition) of g2T^2 via ones-matmul
            sq = sE1.tile([E, R], f32r, tag="sq")
            nc.scalar.activation(sq, g2T_f, AF.Square)
            rn_row = sE1.tile([1, R], f32, tag="rn_row")
            ssb = sE1.tile([1, R], f32, tag="ssb")
            for rh in range(2):
                sl = slice(rh * 512, rh * 512 + 512)
                pss = psE.tile([1, 512], f32, tag="psEsm")
                nc.tensor.matmul(pss, lhsT=ones_r[0:E, :],
                                 rhs=sq[:, rh * 512:rh * 512 + 512], start=True, stop=True)
                nc.scalar.activation(ssb[:, sl], pss, AF.Copy, bias=1e-26)
                nc.vector.reciprocal(rn_row[:, sl], ssb[:, sl])
                nc.scalar.activation(rn_row[:, sl], rn_row[:, sl], AF.Sqrt)
            RN = sE1.tile([E, R], f32, tag="RN")
            nc.gpsimd.partition_broadcast(RN, rn_row)
            zT = sE1.tile([E, R], f32, tag="zT")
            nc.vector.tensor_tensor(zT, g2T_f, RN, op=OP.mult)
            zT_r = sE1.tile([E, R], f32r, tag="zT_r")
            nc.scalar.activation(zT_r, zT, AF.Copy)

            # z rows (output + AllGather input)
            z_sb = sE1.tile([128, RT, E], f32, tag="z_sb")
            for rt in range(RT):
                ptz = psE.tile([128, E], f32, tag="psEsm")
                nc.tensor.transpose(ptz, zT[:, rt * 128:rt * 128 + 128], ident[0:E, 0:E])
                nc.scalar.activation(z_sb[:, rt, :], ptz, AF.Copy)
            nc.sync.dma_start(out=z_re, in_=z_sb)
            nc.sync.dma_start(out=z_b.rearrange("(a p) e -> p a e", p=128), in_=z_sb)
            nc.gpsimd.collective_compute("AllGather", OP.bypass, replica_groups=RG,
                                         ins=[z_b.opt()], outs=[z_full.opt()])

            # ---- q (student-t cluster assignment) ----
            ccT_r = sE1.tile([E, KC], f32r, tag="ccT_r")
            ptc = psE.tile([E, KC], f32, tag="psEsm")
            nc.tensor.transpose(ptc, cc_f, ident[0:KC, 0:KC])
            nc.scalar.activation(ccT_r, ptc, AF.Copy)
            # cc1 = rowsums(cc^2) + 1 -> [1, KC] -> bcast [128, KC]
            cscr = sE1.tile([KC, E], f32, tag="cscr")
            csq = sE1.tile([KC, 1], f32, tag="csq")
            nc.vector.scalar_tensor_tensor(out=cscr, in0=cc_f, scalar=1.0, in1=cc_f,
                                           op0=OP.mult, op1=OP.mult, accum_out=csq)
            ptcs = psE.tile([1, KC], f32, tag="psEsm")
            nc.tensor.transpose(ptcs, csq, ident[0:KC, 0:KC])
            cc1_row = sE1.tile([1, KC], f32, tag="cc1_row")
            nc.scalar.activation(cc1_row, ptcs, AF.Copy, bias=1.0)
            CC1 = sE1.tile([128, KC], f32, tag="CC1")
            nc.gpsimd.partition_broadcast(CC1, cc1_row)

            zz_cols = sE1.tile([128, RT], f32, tag="zz_cols")
            for rt in range(RT):
                zscr = sE.tile([128, E], f32, tag="zscr")
                nc.vector.scalar_tensor_tensor(
                    out=zscr, in0=z_sb[:, rt, :], scalar=1.0, in1=z_sb[:, rt, :],
                    op0=OP.mult, op1=OP.mult, accum_out=zz_cols[:, rt:rt + 1])
            for rt in range(RT):
                pq = psE.tile([128, KC], f32, tag="psEsm")
                nc.tensor.matmul(pq, lhsT=zT_r[:, rt * 128:rt * 128 + 128], rhs=ccT_r,
                                 start=True, stop=True)
                tq = sE.tile([128, KC], f32, tag="tq")
                nc.scalar.activation(tq, pq, AF.Identity, scale=-2.0,
                                     bias=zz_cols[:, rt:rt + 1])
                uq = sE.tile([128, KC], f32, tag="uq")
                nc.vector.tensor_tensor(uq, tq, CC1, op=OP.add)
                wq = sE.tile([128, KC], f32, tag="wq")
                qs = sE.tile([128, 1], f32, tag="qs")
                nc.vector.reciprocal(wq, uq)
                qscr = sE.tile([128, KC], f32, tag="qscr")
                nc.vector.scalar_tensor_tensor(out=qscr, in0=wq, scalar=1.0, in1=wq,
                                               op0=OP.mult, op1=OP.bypass, accum_out=qs)
                rs = sE.tile([128, 1], f32, tag="rs")
                nc.vector.reciprocal(rs, qs)
                qf = sE.tile([128, KC], f32, tag="qf")
                nc.vector.tensor_scalar(out=qf, in0=wq, scalar1=rs[:, 0:1], scalar2=None,
                                        op0=OP.mult)
                nc.sync.dma_start(out=q_re[:, rt, :], in_=qf)

            # ---- A_pred = sigmoid(z @ z^T), row block per rt ----
            zf_sb = sE1.tile([128, 64, E], f32, tag="zf_sb")
            nc.sync.dma_start(out=zf_sb, in_=z_full[:].rearrange("(b p) e -> p b e", p=128))
            zfT = sE1.tile([E, N], f32r, tag="zfT")
            for b in range(0, 64, 4):
                ptf = psE.tile([E, 512], f32, tag="psEptf")
                for k in range(4):
                    nc.tensor.transpose(ptf[:, k * 128:(k + 1) * 128],
                                        zf_sb[:, b + k, :], ident)
                nc.scalar.activation(zfT[:, b * 128:(b + 4) * 128], ptf, AF.Copy)

            for rt in range(RT):
                for half in range(2):
                    stg = sE.tile([128, 8, 512], f32, tag="stg")
                    for jc in range(8):
                        pA = psE.tile([128, 512], f32, tag="psEpA")
                        nc.tensor.matmul(
                            pA, lhsT=zT_r[:, rt * 128:rt * 128 + 128],
                            rhs=zfT[:, (half * 8 + jc) * 512:(half * 8 + jc + 1) * 512],
                            start=True, stop=True)
                        nc.scalar.activation(stg[:, jc, :], pA, AF.Sigmoid)
                    nc.sync.dma_start(
                        out=apred_re[:, rt, half * 4096:half * 4096 + 4096],
                        in_=stg)

    nc.compile()
    return nc


def _get_nc():
    if 'nc' not in _cache:
        _cache['nc'] = _build()
    return _cache['nc']


def kernel(x, adj, M, W1, a_self1, a_neighs1, W2, a_self2, a_neighs2, cluster_centers):
    nc = _get_nc()
    x = np.ascontiguousarray(x, dtype=np.float32)
    adj = np.ascontiguousarray(adj, dtype=np.int32)
    M = np.ascontiguousarray(M, dtype=np.float32)
    shared = {
        "w1": np.ascontiguousarray(W1, dtype=np.float32),
        "a1s": np.ascontiguousarray(a_self1, dtype=np.float32).reshape(1, H),
        "a1n": np.ascontiguousarray(a_neighs1, dtype=np.float32).reshape(1, H),
        "w2": np.ascontiguousarray(W2, dtype=np.float32),
        "a2s": np.ascontiguousarray(a_self2, dtype=np.float32).reshape(1, E),
        "a2n": np.ascontiguousarray(a_neighs2, dtype=np.float32).reshape(1, E),
        "cc": np.ascontiguousarray(cluster_centers, dtype=np.float32),
    }
    in_maps = []
    for c in range(NC):
        sl = slice(c * R, (c + 1) * R)
        in_maps.append({"x": x[sl], "adj": adj[sl], "m": M[sl], **shared})
    res = run_spmd(nc, in_maps)
    A_pred = np.concatenate([res[c]["apred"] for c in range(NC)], axis=0)
    z = np.concatenate([res[c]["z"] for c in range(NC)], axis=0)
    q = np.concatenate([res[c]["q"] for c in range(NC)], axis=0)
    return A_pred, z, q


def run_spmd(nc, in_maps, **kwargs):
    r = bu.run_bass_kernel_spmd(nc, in_maps, core_ids=list(range(NC)), **kwargs)
    _cache['last_result'] = r
    return r.results


# revision 10
# speedup vs baseline: 1.1102x; 1.1102x over previous
"""DAEGC (2-layer GAT + clustering head) forward pass on 8 Trainium2 NeuronCores.

Sharding: row-wise over the N x N attention/adjacency matrices — core i owns
rows [i*1024, (i+1)*1024) of adj, M, attention, h', A_pred.  h1/h2/z are
all-gathered ([N, d], small); W/a/cluster params replicated.

Algorithm notes:
 - softmax without max-subtraction: scores are bounded (|s| < ~25), masked
   entries multiply by adj (0/1) after exp, so masked exp = 0 exactly and
   denominator = sum of masked exps.  h' = (expS @ G) / den.
 - attention aggregation computed transposed: h'^T = G^T @ expS^T with expS
   tiles transposed on the PE; contraction (neighbor j) stays on the
   partition axis with a 512-wide moving free dim.
 - matmul operands are float32r (rounded fp32, ~2e-4 rel) for 4x PE rate.
"""
import sys
sys.path.insert(0, '/opt/trn_rl_repo')
import contextlib
import numpy as np

import concourse.bass as bass
import concourse.tile as tile
import concourse.mybir as mybir
from concourse import bacc
from concourse.masks import make_identity
import concourse.bass_utils as bu

f32 = mybir.dt.float32
f32r = mybir.dt.float32r
i32 = mybir.dt.int32
i8 = mybir.dt.int8
bf16 = mybir.dt.bfloat16
AF = mybir.ActivationFunctionType
OP = mybir.AluOpType

NC = 8            # cores
N = 8192          # nodes
F = 512           # input features
H = 256           # hidden
E = 64            # embed
KC = 16           # clusters
R = N // NC       # rows per core = 1024
RT = R // 128     # row tiles per core = 8
PW = 512          # attention panel width (j)
NPAN = N // PW    # 16 panels
RG = [list(range(NC))]

_cache = {}


def _attention_layer(nc, tc, ctx, consts, out_pool, lay):
    """Emit one GAT attention layer; returns (softmax-normalized + ELU) h'^T
    tile [fp, fcs, 1024] f32 allocated from out_pool."""
    ident, ones_r = consts['ident'], consts['ones_r']
    Fg, tag = lay['Fg'], lay['tag']
    fcs = (Fg + 127) // 128          # feature chunks (2 for L1, 1 for L2)
    fp = min(Fg, 128)                # partitions per chunk
    Gt = lay['G']
    blk = R + lay['aug']             # rows per rank block in augmented gather

    pan_pool = ctx.enter_context(tc.tile_pool(name=f"pan{tag}", bufs=2))
    sc_pool = ctx.enter_context(tc.tile_pool(name=f"sc{tag}", bufs=2))
    nrm_pool = ctx.enter_context(tc.tile_pool(name=f"nrm{tag}", bufs=1))
    et_pool = ctx.enter_context(tc.tile_pool(name=f"et{tag}", bufs=2))
    ps_acc = ctx.enter_context(tc.tile_pool(name=f"psa{tag}", bufs=1, space="PSUM"))
    ps_t = ctx.enter_context(tc.tile_pool(name=f"pst{tag}", bufs=2, space="PSUM"))

    # persistent accumulators: h'^T [fp, 512] per (fc, rh); den [1, 512] per rh
    ph = [[ps_acc.tile([fp, 512], f32, tag=f"ph{fc}{rh}", name=f"ph{tag}{fc}{rh}")
           for rh in range(2)] for fc in range(fcs)]
    pden = [ps_acc.tile([1, 512], f32, tag=f"pden{rh}", name=f"pden{tag}{rh}")
            for rh in range(2)]

    for p in range(NPAN):
        j0 = p * PW
        Mp = pan_pool.tile([128, RT, PW], bf16, tag="Mp")
        Ap = pan_pool.tile([128, RT, PW], i8, tag="Ap")
        Gp = pan_pool.tile([128, 4, Fg], f32r, tag="Gp")
        nc.sync.dma_start(out=Mp, in_=lay['m_re'][:, :, j0:j0 + PW])
        nc.sync.dma_start(out=Ap, in_=lay['adj_re'][:, :, j0:j0 + PW])
        rank, within = j0 // R, j0 % R
        gbase = (rank * blk + within) * Fg
        nc.sync.dma_start(out=Gp, in_=bass.AP(
            tensor=Gt, offset=gbase, ap=[[Fg, 128], [Fg * 128, 4], [1, Fg]]))
        ANp = pan_pool.tile([128, PW], f32, tag="ANp")
        anbase = (rank * blk + R) * Fg + within
        nc.sync.dma_start(out=ANp, in_=bass.AP(tensor=Gt, offset=anbase,
                                               ap=[[0, 128], [1, PW]]).bitcast(f32))

        eT = et_pool.tile([128, 4, 1024], f32r, tag="eT")
        for rt in range(RT):
            t = sc_pool.tile([128, PW], f32, tag="t")
            nc.vector.scalar_tensor_tensor(
                out=t, in0=ANp, scalar=lay['as_cols'][:, rt:rt + 1],
                in1=Mp[:, rt], op0=OP.add, op1=OP.mult)
            v = sc_pool.tile([128, PW], f32, tag="v")
            nc.vector.scalar_tensor_tensor(
                out=v, in0=t, scalar=0.2, in1=t, op0=OP.mult, op1=OP.max)
            e = sc_pool.tile([128, PW], f32, tag="e")
            nc.scalar.activation(e, v, AF.Exp)
            em = sc_pool.tile([128, PW], f32, tag="em")
            nc.vector.scalar_tensor_tensor(
                out=em, in0=e, scalar=1.0, in1=Ap[:, rt],
                op0=OP.mult, op1=OP.mult)
            pt = ps_t.tile([128, 512], f32, tag="pt")
            for jb in range(4):
                nc.tensor.transpose(pt[:, jb * 128:jb * 128 + 128],
                                    em[:, jb * 128:jb * 128 + 128], ident)
            nc.scalar.activation(
                eT[:, :, rt * 128:rt * 128 + 128],
                pt.rearrange("p (b q) -> p b q", b=4), AF.Copy)

        first, last = (p == 0), (p == NPAN - 1)
        for jb in range(4):
            for fc in range(fcs):
                for rh in range(2):
                    nc.tensor.matmul(
                        ph[fc][rh],
                        lhsT=Gp[:, jb, fc * 128:fc * 128 + fp],
                        rhs=eT[:, jb, rh * 512:rh * 512 + 512],
                        start=(first and jb == 0), stop=(last and jb == 3))
            for rh in range(2):
                nc.tensor.matmul(
                    pden[rh], lhsT=ones_r,
                    rhs=eT[:, jb, rh * 512:rh * 512 + 512],
                    start=(first and jb == 0), stop=(last and jb == 3))

    # reciprocal of softmax denominators -> broadcast along feature partitions
    rden_row = nrm_pool.tile([1, 1024], f32, tag="rden_row")
    for rh in range(2):
        nc.vector.reciprocal(rden_row[:, rh * 512:rh * 512 + 512], pden[rh])
    RDEN = nrm_pool.tile([fp, 1024], f32, tag="RDEN")
    nc.gpsimd.partition_broadcast(RDEN, rden_row)

    # normalize (softmax divide) + ELU -> houtT [fp, fcs, 1024] f32
    houtT = out_pool.tile([fp, fcs, 1024], f32, tag=f"houtT{tag}")
    for fc in range(fcs):
        for rh in range(2):
            sl = slice(rh * 512, rh * 512 + 512)
            hn = nrm_pool.tile([fp, 512], f32, tag="hn")
            nc.vector.tensor_tensor(hn, ph[fc][rh], RDEN[:, sl], op=OP.mult)
            mn = nrm_pool.tile([fp, 512], f32, tag="mn")
            nc.vector.tensor_scalar(out=mn, in0=hn, scalar1=0.0, scalar2=None, op0=OP.min)
            ex = nrm_pool.tile([fp, 512], f32, tag="ex")
            nc.scalar.activation(ex, mn, AF.Exp)
            rl = nrm_pool.tile([fp, 512], f32, tag="rl")
            nc.vector.tensor_relu(rl, hn)
            nc.vector.scalar_tensor_tensor(
                out=houtT[:, fc, sl], in0=ex, scalar=-1.0, in1=rl,
                op0=OP.add, op1=OP.add)
    return houtT


def _build():
    nc = bacc.Bacc("TRN2", target_bir_lowering=False, debug=False, num_devices=NC)

    x_d = nc.dram_tensor("x", [R, F], f32, kind="ExternalInput")
    adj_d = nc.dram_tensor("adj", [R, N], i8, kind="ExternalInput")
    m_d = nc.dram_tensor("m", [R, N], bf16, kind="ExternalInput")
    w1_d = nc.dram_tensor("w1", [F, H], f32r, kind="ExternalInput")
    a1s_d = nc.dram_tensor("a1s", [1, H], f32r, kind="ExternalInput")
    a1n_d = nc.dram_tensor("a1n", [1, H], f32r, kind="ExternalInput")
    w2_d = nc.dram_tensor("w2", [H, E], f32r, kind="ExternalInput")
    a2s_d = nc.dram_tensor("a2s", [1, E], f32r, kind="ExternalInput")
    a2n_d = nc.dram_tensor("a2n", [1, E], f32r, kind="ExternalInput")
    cc_d = nc.dram_tensor("cc", [KC, E], f32, kind="ExternalInput")

    apred_d = nc.dram_tensor("apred", [R, N], f32, kind="ExternalOutput")
    z_d = nc.dram_tensor("z", [R, E], f32, kind="ExternalOutput")
    q_d = nc.dram_tensor("q", [R, KC], f32, kind="ExternalOutput")

    adj_re = adj_d.ap().rearrange("(a p) j -> p a j", p=128)
    m_re = m_d.ap().rearrange("(a p) j -> p a j", p=128)
    x_re = x_d.ap().rearrange("(a p) f -> p a f", p=128)
    apred_re = apred_d.ap().rearrange("(a p) j -> p a j", p=128)
    z_re = z_d.ap().rearrange("(a p) e -> p a e", p=128)
    q_re = q_d.ap().rearrange("(a p) k -> p a k", p=128)

    with tile.TileContext(nc, num_cores=NC) as tc, contextlib.ExitStack() as top:
        const = top.enter_context(tc.tile_pool(name="const", bufs=1))
        dram = top.enter_context(tc.tile_pool(name="dram", bufs=1, space="DRAM"))
        keep = top.enter_context(tc.tile_pool(name="keep", bufs=1))

        ident = const.tile([128, 128], f32)
        make_identity(nc, ident)
        ones_f = const.tile([128, 1], f32)
        nc.vector.memset(ones_f, 1.0)
        ones_r = const.tile([128, 1], f32r)
        nc.scalar.activation(ones_r, ones_f, AF.Copy)
        consts = dict(ident=ident, ones_r=ones_r)

        # ---- params to SBUF (f32r for matmul operands) ----
        w1_r = const.tile([128, 4, H], f32r)
        nc.sync.dma_start(out=w1_r, in_=w1_d.ap().rearrange("(c p) h -> p c h", p=128))
        w2_r = const.tile([128, 2, E], f32r)
        nc.sync.dma_start(out=w2_r, in_=w2_d.ap().rearrange("(c p) e -> p c e", p=128))

        def vec_cols(d, n):  # DRAM [1, n] -> SBUF column layout [min(n,128), n/p] f32r
            p = min(n, 128)
            c = n // p
            r = const.tile([p, c], f32r, tag=f"vcr{d.name}")
            nc.sync.dma_start(out=r, in_=bass.AP(tensor=d, offset=0, ap=[[1, p], [p, c]]))
            return r

        a1s_cols = vec_cols(a1s_d, H)   # [128, 2]
        a1n_cols = vec_cols(a1n_d, H)   # [128, 2]
        a2s_cols = vec_cols(a2s_d, E)   # [64, 1]
        a2n_cols = vec_cols(a2n_d, E)   # [64, 1]

        cc_f = const.tile([KC, E], f32)
        nc.sync.dma_start(out=cc_f, in_=cc_d.ap())

        # DRAM bounces for collectives.  h bounces carry an (neighbor scores)
        # as R/Fg extra rows so one AllGather moves both.
        AUG1 = R // H   # 4 extra rows for layer 1
        AUG2 = R // E   # 16 extra rows for layer 2
        h1_b = dram.tile([R + AUG1, H], f32r)
        h1_full = dram.tile([NC * (R + AUG1), H], f32r, addr_space="Shared")
        h2_b = dram.tile([R + AUG2, E], f32r)
        h2_full = dram.tile([NC * (R + AUG2), E], f32r, addr_space="Shared")
        z_b = dram.tile([R, E], f32)
        z_full = dram.tile([N, E], f32, addr_space="Shared")

        # ================= stage A: h1 = x @ W1, attention vectors =================
        with contextlib.ExitStack() as sa:
            sA = sa.enter_context(tc.tile_pool(name="sA", bufs=2))
            psA = sa.enter_context(tc.tile_pool(name="psA", bufs=2, space="PSUM"))

            xs = sA.tile([128, RT, F], f32, tag="xs")
            nc.sync.dma_start(out=xs, in_=x_re)
            xT = sA.tile([128, 4, R], f32r, tag="xT")
            for rt in range(RT):
                ptx = psA.tile([128, 512], f32, tag="psAbig")
                for fc in range(4):
                    nc.tensor.transpose(ptx[:, fc * 128:(fc + 1) * 128],
                                        xs[:, rt, fc * 128:(fc + 1) * 128], ident)
                nc.scalar.activation(xT[:, :, rt * 128:rt * 128 + 128],
                                     ptx.rearrange("p (c q) -> p c q", c=4), AF.Copy)

            h1T = sA.tile([128, 2, R], f32r, tag="h1T")
            for hh in range(2):
                for rh in range(2):
                    ph1 = psA.tile([128, 512], f32, tag="psAbig")
                    for fc in range(4):
                        nc.tensor.matmul(ph1, lhsT=w1_r[:, fc, hh * 128:(hh + 1) * 128],
                                         rhs=xT[:, fc, rh * 512:rh * 512 + 512],
                                         start=(fc == 0), stop=(fc == 3))
                    nc.scalar.activation(h1T[:, hh, rh * 512:rh * 512 + 512], ph1, AF.Copy)

            # h1 row-major (for AllGather) — f32r
            h1_sb = sA.tile([128, RT, H], f32r, tag="h1_sb")
            for rt in range(RT):
                pth = psA.tile([128, 256], f32, tag="psAbig")
                for hh in range(2):
                    nc.tensor.transpose(pth[:, hh * 128:(hh + 1) * 128],
                                        h1T[:, hh, rt * 128:rt * 128 + 128].bitcast(f32), ident)
                nc.scalar.activation(h1_sb[:, rt, :],
                                     pth.rearrange("p (c q) -> p c q", c=2), AF.Copy)
            nc.sync.dma_start(out=h1_b[0:R, :].rearrange("(a p) h -> p a h", p=128), in_=h1_sb)

            # an1/as1 rows [1, R] via matmul, as1 -> columns via PE transpose
            an1_row = sA.tile([1, R], f32, tag="an1_row")
            as1_row = sA.tile([1, R], f32, tag="as1_row")
            for (vec, row) in ((a1n_cols, an1_row), (a1s_cols, as1_row)):
                for rh in range(2):
                    pa = psA.tile([1, 512], f32, tag="psAsm")
                    for hh in range(2):
                        nc.tensor.matmul(pa, lhsT=vec[:, hh:hh + 1],
                                         rhs=h1T[:, hh, rh * 512:rh * 512 + 512],
                                         start=(hh == 0), stop=(hh == 1))
                    nc.scalar.activation(row[:, rh * 512:rh * 512 + 512], pa, AF.Copy)
            nc.sync.dma_start(out=h1_b[R:R + 4, :], in_=an1_row.bitcast(f32r))

            as1_cols = keep.tile([128, RT], f32, tag="as1_cols")
            pas = psA.tile([128, RT], f32, tag="psAsm")
            for rt in range(RT):
                nc.tensor.transpose(pas[:, rt:rt + 1],
                                    as1_row[0:1, rt * 128:rt * 128 + 128], ident[0:1, 0:1])
            nc.scalar.activation(as1_cols, pas, AF.Copy)

            nc.gpsimd.collective_compute("AllGather", OP.bypass, replica_groups=RG,
                                         ins=[h1_b.opt()], outs=[h1_full.opt()])


        # ================= stage B: layer-1 attention =================
        with contextlib.ExitStack() as sb1:
            sb1_out = sb1.enter_context(tc.tile_pool(name="ob1", bufs=1))
            x2T = _attention_layer(nc, tc, sb1, consts, sb1_out, dict(
                tag="L1", G=h1_full.tensor, aug=4, as_cols=as1_cols,
                m_re=m_re, adj_re=adj_re, Fg=H))
            x2T_r = keep.tile([128, 2, R], f32r, tag="x2T_r")
            nc.scalar.activation(x2T_r, x2T, AF.Copy)

        # ================= stage C: h2 = x2 @ W2, attention vectors =================
        with contextlib.ExitStack() as sc:
            sC = sc.enter_context(tc.tile_pool(name="sC", bufs=2))
            psC = sc.enter_context(tc.tile_pool(name="psC", bufs=2, space="PSUM"))

            h2T = sC.tile([E, R], f32r, tag="h2T")
            for rh in range(2):
                ph2 = psC.tile([E, 512], f32, tag="psCbig")
                for fc in range(2):
                    nc.tensor.matmul(ph2, lhsT=w2_r[:, fc, :],
                                     rhs=x2T_r[:, fc, rh * 512:rh * 512 + 512],
                                     start=(fc == 0), stop=(fc == 1))
                nc.scalar.activation(h2T[:, rh * 512:rh * 512 + 512], ph2, AF.Copy)

            h2_sb = sC.tile([128, RT, E], f32r, tag="h2_sb")
            for rt in range(RT):
                pth2 = psC.tile([128, E], f32, tag="psCbig")
                nc.tensor.transpose(pth2, h2T[:, rt * 128:rt * 128 + 128].bitcast(f32), ident[0:E, 0:E])
                nc.scalar.activation(h2_sb[:, rt, :], pth2, AF.Copy)
            nc.sync.dma_start(out=h2_b[0:R, :].rearrange("(a p) e -> p a e", p=128), in_=h2_sb)

            an2_row = sC.tile([1, R], f32, tag="an2_row")
            as2_row = sC.tile([1, R], f32, tag="as2_row")
            for (vec, row) in ((a2n_cols, an2_row), (a2s_cols, as2_row)):
                for rh in range(2):
                    pa2 = psC.tile([1, 512], f32, tag="psCsm")
                    nc.tensor.matmul(pa2, lhsT=vec, rhs=h2T[:, rh * 512:rh * 512 + 512],
                                     start=True, stop=True)
                    nc.scalar.activation(row[:, rh * 512:rh * 512 + 512], pa2, AF.Copy)
            nc.sync.dma_start(out=h2_b[R:R + 16, :], in_=an2_row.bitcast(f32r))

            as2_cols = keep.tile([128, RT], f32, tag="as2_cols")
            pas2 = psC.tile([128, RT], f32, tag="psCsm")
            for rt in range(RT):
                nc.tensor.transpose(pas2[:, rt:rt + 1],
                                    as2_row[0:1, rt * 128:rt * 128 + 128], ident[0:1, 0:1])
            nc.scalar.activation(as2_cols, pas2, AF.Copy)

            nc.gpsimd.collective_compute("AllGather", OP.bypass, replica_groups=RG,
                                         ins=[h2_b.opt()], outs=[h2_full.opt()])


        # ================= stage D: layer-2 attention =================
        with contextlib.ExitStack() as sb2:
            g2T = _attention_layer(nc, tc, sb2, consts, keep, dict(
                tag="L2", G=h2_full.tensor, aug=16, as_cols=as2_cols,
                m_re=m_re, adj_re=adj_re, Fg=E))
        g2T_f = g2T.rearrange("p one r -> p (one r)")  # [64, 1024]

        # ================= stage E: z, q, A_pred =================
        with contextlib.ExitStack() as se:
            sE = se.enter_context(tc.tile_pool(name="sE", bufs=2))
            sE1 = se.enter_context(tc.tile_pool(name="sE1", bufs=1))
            psE = se.enter_context(tc.tile_pool(name="psE", bufs=2, space="PSUM"))

            # row L2 norms: sum over e (partition) of g2T^2 via ones-matmul
            sq = sE1.tile([E, R], f32r, tag="sq")
            nc.scalar.activation(sq, g2T_f, AF.Square)
            rn_row = sE1.tile([1, R], f32, tag="rn_row")
            ssb = sE1.tile([1, R], f32, tag="ssb")
            for rh in range(2):
                sl = slice(rh * 512, rh * 512 + 512)
                pss = psE.tile([1, 512], f32, tag="psEsm")
                nc.tensor.matmul(pss, lhsT=ones_r[0:E, :],
                                 rhs=sq[:, rh * 512:rh * 512 + 512], start=True, stop=True)
                nc.scalar.activation(ssb[:, sl], pss, AF.Copy, bias=1e-26)
                nc.vector.reciprocal(rn_row[:, sl], ssb[:, sl])
                nc.scalar.activation(rn_row[:, sl], rn_row[:, sl], AF.Sqrt)
            RN = sE1.tile([E, R], f32, tag="RN")
            nc.gpsimd.partition_broadcast(RN, rn_row)
            zT = sE1.tile([E, R], f32, tag="zT")
            nc.vector.tensor_tensor(zT, g2T_f, RN, op=OP.mult)
            zT_r = sE1.tile([E, R], f32r, tag="zT_r")
            nc.scalar.activation(zT_r, zT, AF.Copy)

            # z rows (output + AllGather input)
            z_sb = sE1.tile([128, RT, E], f32, tag="z_sb")
            for rt in range(RT):
                ptz = psE.tile([128, E], f32, tag="psEsm")
                nc.tensor.transpose(ptz, zT[:, rt * 128:rt * 128 + 128], ident[0:E, 0:E])
                nc.scalar.activation(z_sb[:, rt, :], ptz, AF.Copy)
            nc.sync.dma_start(out=z_re, in_=z_sb)
            nc.sync.dma_start(out=z_b.rearrange("(a p) e -> p a e", p=128), in_=z_sb)
            nc.gpsimd.collective_compute("AllGather", OP.bypass, replica_groups=RG,
                                         ins=[z_b.opt()], outs=[z_full.opt()])

            # ---- q (student-t cluster assignment) ----
            ccT_r = sE1.tile([E, KC], f32r, tag="ccT_r")
            ptc = psE.tile([E, KC], f32, tag="psEsm")
            nc.tensor.transpose(ptc, cc_f, ident[0:KC, 0:KC])
            nc.scalar.activation(ccT_r, ptc, AF.Copy)
            # cc1 = rowsums(cc^2) + 1 -> [1, KC] -> bcast [128, KC]
            cscr = sE1.tile([KC, E], f32, tag="cscr")
            csq = sE1.tile([KC, 1], f32, tag="csq")
            nc.vector.scalar_tensor_tensor(out=cscr, in0=cc_f, scalar=1.0, in1=cc_f,
                                           op0=OP.mult, op1=OP.mult, accum_out=csq)
            ptcs = psE.tile([1, KC], f32, tag="psEsm")
            nc.tensor.transpose(ptcs, csq, ident[0:KC, 0:KC])
            cc1_row = sE1.tile([1, KC], f32, tag="cc1_row")
            nc.scalar.activation(cc1_row, ptcs, AF.Copy, bias=1.0)
            CC1 = sE1.tile([128, KC], f32, tag="CC1")
            nc.gpsimd.partition_broadcast(CC1, cc1_row)

            zz_cols = sE1.tile([128, RT], f32, tag="zz_cols")
            for rt in range(RT):
                zscr = sE.tile([128, E], f32, tag="zscr")
                nc.vector.scalar_tensor_tensor(
                    out=zscr, in0=z_sb[:, rt, :], scalar=1.0, in1=z_sb[:, rt, :],
                    op0=OP.mult, op1=OP.mult, accum_out=zz_cols[:, rt:rt + 1])
            for rt in range(RT):
                pq = psE.tile([128, KC], f32, tag="psEsm")
                nc.tensor.matmul(pq, lhsT=zT_r[:, rt * 128:rt * 128 + 128], rhs=ccT_r,
                                 start=True, stop=True)
                tq = sE.tile([128, KC], f32, tag="tq")
                nc.scalar.activation(tq, pq, AF.Identity, scale=-2.0,
                                     bias=zz_cols[:, rt:rt + 1])
                uq = sE.tile([128, KC], f32, tag="uq")
                nc.vector.tensor_tensor(uq, tq, CC1, op=OP.add)
                wq = sE.tile([128, KC], f32, tag="wq")
                qs = sE.tile([128, 1], f32, tag="qs")
                nc.vector.reciprocal(wq, uq)
                qscr = sE.tile([128, KC], f32, tag="qscr")
                nc.vector.scalar_tensor_tensor(out=qscr, in0=wq, scalar=1.0, in1=wq,
                                               op0=OP.mult, op1=OP.bypass, accum_out=qs)
                rs = sE.tile([128, 1], f32, tag="rs")
                nc.vector.reciprocal(rs, qs)
                qf = sE.tile([128, KC], f32, tag="qf")
                nc.vector.tensor_scalar(out=qf, in0=wq, scalar1=rs[:, 0:1], scalar2=None,
                                        op0=OP.mult)
                nc.sync.dma_start(out=q_re[:, rt, :], in_=qf)

            # ---- A_pred = sigmoid(z @ z^T), row block per rt ----
            zf_sb = sE1.tile([128, 64, E], f32, tag="zf_sb")
            nc.sync.dma_start(out=zf_sb, in_=z_full[:].rearrange("(b p) e -> p b e", p=128))
            zfT = sE1.tile([E, N], f32r, tag="zfT")
            for b in range(0, 64, 4):
                ptf = psE.tile([E, 512], f32, tag="psEptf")
                for k in range(4):
                    nc.tensor.transpose(ptf[:, k * 128:(k + 1) * 128],
                                        zf_sb[:, b + k, :], ident)
                nc.scalar.activation(zfT[:, b * 128:(b + 4) * 128], ptf, AF.Copy)

            for rt in range(RT):
                for half in range(2):
                    stg = sE.tile([128, 8, 512], f32, tag="stg")
                    for jc in range(8):
                        pA = psE.tile([128, 512], f32, tag="psEpA")
                        nc.tensor.matmul(
                            pA, lhsT=zT_r[:, rt * 128:rt * 128 + 128],
                            rhs=zfT[:, (half * 8 + jc) * 512:(half * 8 + jc + 1) * 512],
                            start=True, stop=True)
                        nc.scalar.activation(stg[:, jc, :], pA, AF.Sigmoid)
                    nc.sync.dma_start(
                        out=apred_re[:, rt, half * 4096:half * 4096 + 4096],
                        in_=stg)

    nc.compile()
    return nc


def _get_nc():
    if 'nc' not in _cache:
        _cache['nc'] = _build()
    return _cache['nc']


def kernel(x, adj, M, W1, a_self1, a_neighs1, W2, a_self2, a_neighs2, cluster_centers):
    nc = _get_nc()
    import ml_dtypes
    x = np.ascontiguousarray(x, dtype=np.float32)
    adj = np.ascontiguousarray(adj).astype(np.int8)
    M = np.ascontiguousarray(M, dtype=np.float32).astype(ml_dtypes.bfloat16)
    shared = {
        "w1": np.ascontiguousarray(W1, dtype=np.float32),
        "a1s": np.ascontiguousarray(a_self1, dtype=np.float32).reshape(1, H),
        "a1n": np.ascontiguousarray(a_neighs1, dtype=np.float32).reshape(1, H),
        "w2": np.ascontiguousarray(W2, dtype=np.float32),
        "a2s": np.ascontiguousarray(a_self2, dtype=np.float32).reshape(1, E),
        "a2n": np.ascontiguousarray(a_neighs2, dtype=np.float32).reshape(1, E),
        "cc": np.ascontiguousarray(cluster_centers, dtype=np.float32),
    }
    in_maps = []
    for c in range(NC):
        sl = slice(c * R, (c + 1) * R)
        in_maps.append({"x": x[sl], "adj": adj[sl], "m": M[sl], **shared})
    res = run_spmd(nc, in_maps)
    A_pred = np.concatenate([res[c]["apred"] for c in range(NC)], axis=0)
    z = np.concatenate([res[c]["z"] for c in range(NC)], axis=0)
    q = np.concatenate([res[c]["q"] for c in range(NC)], axis=0)
    return A_pred, z, q


def run_spmd(nc, in_maps, **kwargs):
    r = bu.run_bass_kernel_spmd(nc, in_maps, core_ids=list(range(NC)), **kwargs)
    _cache['last_result'] = r
    return r.results
